# revision 1
# baseline (speedup 1.0000x reference)
"""Trainium2 Bass kernel for nn_CPCircuitLayer.

Math: with all_indices the full cartesian grid (s = n // H, h = n % H),
    out[b, s, h] = sum_r seq_emb[b,s,r] * hid_emb[b,h,r] * cp[r]
                 = (seq_emb[b] @ diag(cp) @ hid_emb[b].T)[s, h]
where seq_emb[b] = X_b @ seq_W.T  (X_b = hidden_states[b], contract H)
      hid_emb[b] = X_b.T @ hid_W.T                        (contract S)

Sharding: 8 cores = (batch b, seq half) pairs. Each core holds X_b fully
(needed for the hid factor) plus a host-transposed copy of its own seq
half (X_b[half].T, needed for the seq factor), computes
    hid_embT = (hid_W*cp) @ X_b          [R, H]
    seq_embT = seq_W @ X_b[half].T       [R, S/2]
    out_half = seq_embT.T @ hid_embT     [S/2, H]
and writes its [512, 1024] slice of the output.
"""

import numpy as np

B, S, H, R = 4, 1024, 1024, 32
N_CORES = 8
SH = S // 2  # seq rows per core

_compiled = {}


def _np_fallback(hidden_states, all_indices, seq_W, hid_W, cp_weight):
    seq_emb = np.einsum("bsh,rh->bsr", hidden_states, seq_W)
    hid_emb = np.einsum("bsh,rs->bhr", hidden_states, hid_W)
    s_idx = all_indices[:, 0].astype(np.int64)
    h_idx = all_indices[:, 1].astype(np.int64)
    g_seq = seq_emb[:, s_idx, :]
    g_hid = hid_emb[:, h_idx, :]
    out = np.einsum("bnr,bnr,r->bn", g_seq, g_hid, cp_weight[0])
    return out.reshape(B, S, H).astype(np.float32)


def _build_program():
    import concourse.mybir as mybir
    import concourse.tile as tile
    from concourse import bacc

    f32 = mybir.dt.float32

    nc = bacc.Bacc("TRN2", target_bir_lowering=False, debug=False,
                   num_devices=N_CORES)

    x_d = nc.dram_tensor("x", [S, H], f32, kind="ExternalInput")
    xt_d = nc.dram_tensor("xt", [H, SH], f32, kind="ExternalInput")
    sw_d = nc.dram_tensor("sw", [H, R], f32, kind="ExternalInput")
    hw_d = nc.dram_tensor("hw", [S, R], f32, kind="ExternalInput")
    out_d = nc.dram_tensor("out", [SH, H], f32, kind="ExternalOutput")

    KT = S // 128  # 8 k-tiles over the contraction dims

    with tile.TileContext(nc) as tc:
        with (
            tc.tile_pool(name="xp", bufs=1) as xp,
            tc.tile_pool(name="wp", bufs=1) as wp,
            tc.tile_pool(name="fp", bufs=1) as fp,
            tc.tile_pool(name="op", bufs=3) as op,
            tc.tile_pool(name="fps", bufs=1, space="PSUM") as fps,
            tc.tile_pool(name="ops", bufs=3, space="PSUM") as ops,
        ):
            # weights first (small, needed by every factor matmul)
            sw_t = wp.tile([128, KT, R], f32)
            nc.sync.dma_start(sw_t[:], sw_d.rearrange("(t p) r -> p t r", p=128))
            hw_t = wp.tile([128, KT, R], f32)
            nc.sync.dma_start(hw_t[:], hw_d.rearrange("(t p) r -> p t r", p=128))

            x_t = []
            for k in range(KT):
                xk = xp.tile([128, H], f32, name=f"x{k}")
                nc.sync.dma_start(xk[:], x_d[k * 128:(k + 1) * 128, :])
                x_t.append(xk)
            xt_t = []
            for k in range(KT):
                xtk = xp.tile([128, SH], f32, name=f"xt{k}")
                nc.sync.dma_start(xtk[:], xt_d[k * 128:(k + 1) * 128, :])
                xt_t.append(xtk)

            # hid_embT[r, h] = sum_s hid_wT[s, r] * x[s, h]
            hid_sb = fp.tile([R, H], f32)
            for n in range(2):
                hid_ps = fps.tile([R, 512], f32, name="hid_ps", bufs=2)
                for k in range(KT):
                    nc.tensor.matmul(
                        hid_ps[:],
                        hw_t[:, k, :],
                        x_t[k][:, n * 512:(n + 1) * 512],
                        start=(k == 0),
                        stop=(k == KT - 1),
                    )
                nc.vector.tensor_copy(hid_sb[:, n * 512:(n + 1) * 512], hid_ps[:])

            # seq_embT[r, s] = sum_h seq_wT[h, r] * xt[h, s]
            seq_ps = fps.tile([R, SH], f32)
            for k in range(KT):
                nc.tensor.matmul(
                    seq_ps[:],
                    sw_t[:, k, :],
                    xt_t[k][:],
                    start=(k == 0),
                    stop=(k == KT - 1),
                )
            seq_sb = fp.tile([R, SH], f32)
            nc.vector.tensor_copy(seq_sb[:], seq_ps[:])

            # out[s, h] = sum_r seq_embT[r, s] * hid_embT[r, h]
            for m in range(SH // 128):
                o_sb = op.tile([128, H], f32, name="o_sb")
                for n in range(2):
                    o_ps = ops.tile([128, 512], f32, name="o_ps")
                    nc.tensor.matmul(
                        o_ps[:],
                        seq_sb[:, m * 128:(m + 1) * 128],
                        hid_sb[:, n * 512:(n + 1) * 512],
                    )
                    nc.vector.tensor_copy(o_sb[:, n * 512:(n + 1) * 512], o_ps[:])
                nc.sync.dma_start(out_d[m * 128:(m + 1) * 128, :], o_sb[:])

    nc.compile()
    return nc


def _get_program():
    if "nc" not in _compiled:
        _compiled["nc"] = _build_program()
    return _compiled["nc"]


def kernel(hidden_states, all_indices, seq_W, hid_W, cp_weight):
    hidden_states = np.asarray(hidden_states, dtype=np.float32)
    seq_W = np.asarray(seq_W, dtype=np.float32)
    hid_W = np.asarray(hid_W, dtype=np.float32)
    cp_weight = np.asarray(cp_weight, dtype=np.float32)
    idx = np.asarray(all_indices)

    # The reference's all_indices is always the full cartesian grid; verify
    # cheaply and fall back to a host path if ever not.
    n = np.arange(S * H, dtype=idx.dtype)
    if idx.shape != (S * H, 2) or not (
        np.array_equal(idx[:, 0], n // H) and np.array_equal(idx[:, 1], n % H)
    ):
        return _np_fallback(hidden_states, idx, seq_W, hid_W, cp_weight)

    from concourse.bass_utils import run_bass_kernel_spmd

    nc = _get_program()

    swT = np.ascontiguousarray(seq_W.T)                     # [H, R]
    hwT = np.ascontiguousarray((hid_W * cp_weight[0][:, None]).T)  # [S, R]

    in_maps = []
    for c in range(N_CORES):
        b, half = divmod(c, 2)
        xb = hidden_states[b]
        in_maps.append({
            "x": xb,
            "xt": np.ascontiguousarray(xb[half * SH:(half + 1) * SH, :].T),
            "sw": swT,
            "hw": hwT,
        })

    res = run_bass_kernel_spmd(nc, in_maps, list(range(N_CORES)))

    out = np.empty((B, S, H), dtype=np.float32)
    for c in range(N_CORES):
        b, half = divmod(c, 2)
        out[b, half * SH:(half + 1) * SH, :] = res.results[c]["out"]
    return out


# revision 4
# speedup vs baseline: 1.2202x; 1.2202x over previous
"""Trainium2 Bass kernel for nn_CPCircuitLayer.

Math: with all_indices the full cartesian grid (s = n // H, h = n % H),
    out[b, s, h] = sum_r seq_emb[b,s,r] * hid_emb[b,h,r] * cp[r]
                 = (seq_emb[b] @ diag(cp) @ hid_emb[b].T)[s, h]
where seq_emb[b] = X_b @ seq_W.T  (X_b = hidden_states[b], contract H)
      hid_emb[b] = X_b.T @ hid_W.T                        (contract S)

Sharding: 8 cores = (batch b, seq half) pairs. Each core holds X_b fully
(needed for the hid factor) plus a host-transposed copy of its own seq
half (X_b[half].T, needed for the seq factor), computes
    hid_embT = (hid_W*cp) @ X_b          [R, H]
    seq_embT = seq_W @ X_b[half].T       [R, S/2]
    out_half = seq_embT.T @ hid_embT     [S/2, H]
and writes its [512, 1024] slice of the output.
"""

import numpy as np

B, S, H, R = 4, 1024, 1024, 32
N_CORES = 8
SH = S // 2  # seq rows per core

_compiled = {}


def _np_fallback(hidden_states, all_indices, seq_W, hid_W, cp_weight):
    seq_emb = np.einsum("bsh,rh->bsr", hidden_states, seq_W)
    hid_emb = np.einsum("bsh,rs->bhr", hidden_states, hid_W)
    s_idx = all_indices[:, 0].astype(np.int64)
    h_idx = all_indices[:, 1].astype(np.int64)
    g_seq = seq_emb[:, s_idx, :]
    g_hid = hid_emb[:, h_idx, :]
    out = np.einsum("bnr,bnr,r->bn", g_seq, g_hid, cp_weight[0])
    return out.reshape(B, S, H).astype(np.float32)


def _build_program(use_f32r=True):
    import concourse.mybir as mybir
    import concourse.tile as tile
    from concourse import bacc

    f32 = mybir.dt.float32
    mmdt = mybir.dt.float32r if use_f32r else f32

    nc = bacc.Bacc("TRN2", target_bir_lowering=False, debug=False,
                   num_devices=N_CORES)

    x_d = nc.dram_tensor("x", [S, H], f32, kind="ExternalInput")
    xt_d = nc.dram_tensor("xt", [H, SH], f32, kind="ExternalInput")
    sw_d = nc.dram_tensor("sw", [H, R], f32, kind="ExternalInput")
    hw_d = nc.dram_tensor("hw", [S, R], f32, kind="ExternalInput")
    out_d = nc.dram_tensor("out", [SH, H], f32, kind="ExternalOutput")

    KT = S // 128  # 8 k-tiles over the contraction dims

    with tile.TileContext(nc) as tc:
        with (
            tc.tile_pool(name="xp", bufs=1) as xp,
            tc.tile_pool(name="wp", bufs=1) as wp,
            tc.tile_pool(name="fp", bufs=1) as fp,
            tc.tile_pool(name="op", bufs=3) as op,
            tc.tile_pool(name="fps", bufs=1, space="PSUM") as fps,
            tc.tile_pool(name="ops", bufs=3, space="PSUM") as ops,
        ):
            def load(name, dram_ap, shape):
                """DMA f32 from DRAM; convert to the matmul dtype if needed."""
                raw = xp.tile(shape, f32, name=name)
                nc.sync.dma_start(raw[:], dram_ap)
                if not use_f32r:
                    return raw
                conv = xp.tile(shape, mmdt, name=name + "r")
                nc.vector.tensor_copy(conv[:], raw[:])
                return conv

            # weights first (small, needed by every factor matmul)
            sw_t = load("sw", sw_d.rearrange("(t p) r -> p t r", p=128),
                        [128, KT, R])
            hw_t = load("hw", hw_d.rearrange("(t p) r -> p t r", p=128),
                        [128, KT, R])

            x_t = [load(f"x{k}", x_d[k * 128:(k + 1) * 128, :], [128, H])
                   for k in range(KT)]
            xt_t = [load(f"xt{k}", xt_d[k * 128:(k + 1) * 128, :], [128, SH])
                    for k in range(KT)]

            # hid_embT[r, h] = sum_s hid_wT[s, r] * x[s, h]
            hid_sb = fp.tile([R, H], mmdt)
            for n in range(2):
                hid_ps = fps.tile([R, 512], f32, name="hid_ps", bufs=2)
                for k in range(KT):
                    nc.tensor.matmul(
                        hid_ps[:],
                        hw_t[:, k, :],
                        x_t[k][:, n * 512:(n + 1) * 512],
                        start=(k == 0),
                        stop=(k == KT - 1),
                    )
                nc.vector.tensor_copy(hid_sb[:, n * 512:(n + 1) * 512], hid_ps[:])

            # seq_embT[r, s] = sum_h seq_wT[h, r] * xt[h, s]
            seq_ps = fps.tile([R, SH], f32)
            for k in range(KT):
                nc.tensor.matmul(
                    seq_ps[:],
                    sw_t[:, k, :],
                    xt_t[k][:],
                    start=(k == 0),
                    stop=(k == KT - 1),
                )
            seq_sb = fp.tile([R, SH], mmdt)
            nc.vector.tensor_copy(seq_sb[:], seq_ps[:])

            # out[s, h] = sum_r seq_embT[r, s] * hid_embT[r, h]
            for m in range(SH // 128):
                o_sb = op.tile([128, H], f32, name="o_sb")
                for n in range(2):
                    o_ps = ops.tile([128, 512], f32, name="o_ps")
                    nc.tensor.matmul(
                        o_ps[:],
                        seq_sb[:, m * 128:(m + 1) * 128],
                        hid_sb[:, n * 512:(n + 1) * 512],
                    )
                    nc.vector.tensor_copy(o_sb[:, n * 512:(n + 1) * 512], o_ps[:])
                nc.sync.dma_start(out_d[m * 128:(m + 1) * 128, :], o_sb[:])

    nc.compile()
    return nc


def _get_program():
    if "nc" not in _compiled:
        _compiled["nc"] = _build_program()
    return _compiled["nc"]


def _make_in_maps(hidden_states, seq_W, hid_W, cp_weight):
    swT = np.ascontiguousarray(seq_W.T)                            # [H, R]
    hwT = np.ascontiguousarray((hid_W * cp_weight[0][:, None]).T)  # [S, R]
    in_maps = []
    for c in range(N_CORES):
        b, half = divmod(c, 2)
        xb = hidden_states[b]
        in_maps.append({
            "x": xb,
            "xt": np.ascontiguousarray(xb[half * SH:(half + 1) * SH, :].T),
            "sw": swT,
            "hw": hwT,
        })
    return in_maps


def kernel(hidden_states, all_indices, seq_W, hid_W, cp_weight):
    hidden_states = np.asarray(hidden_states, dtype=np.float32)
    seq_W = np.asarray(seq_W, dtype=np.float32)
    hid_W = np.asarray(hid_W, dtype=np.float32)
    cp_weight = np.asarray(cp_weight, dtype=np.float32)
    idx = np.asarray(all_indices)

    # The reference's all_indices is always the full cartesian grid; verify
    # cheaply and fall back to a host path if ever not.
    n = np.arange(S * H, dtype=idx.dtype)
    if idx.shape != (S * H, 2) or not (
        np.array_equal(idx[:, 0], n // H) and np.array_equal(idx[:, 1], n % H)
    ):
        return _np_fallback(hidden_states, idx, seq_W, hid_W, cp_weight)

    from concourse.bass_utils import run_bass_kernel_spmd

    nc = _get_program()
    in_maps = _make_in_maps(hidden_states, seq_W, hid_W, cp_weight)
    res = run_bass_kernel_spmd(nc, in_maps, list(range(N_CORES)))

    out = np.empty((B, S, H), dtype=np.float32)
    for c in range(N_CORES):
        b, half = divmod(c, 2)
        out[b, half * SH:(half + 1) * SH, :] = res.results[c]["out"]
    return out


# revision 5
# speedup vs baseline: 1.2619x; 1.0342x over previous
"""Trainium2 Bass kernel for nn_CPCircuitLayer.

Math: with all_indices the full cartesian grid (s = n // H, h = n % H),
    out[b, s, h] = sum_r seq_emb[b,s,r] * hid_emb[b,h,r] * cp[r]
                 = (seq_emb[b] @ diag(cp) @ hid_emb[b].T)[s, h]
where seq_emb[b] = X_b @ seq_W.T  (X_b = hidden_states[b], contract H)
      hid_emb[b] = X_b.T @ hid_W.T                        (contract S)

Sharding: 8 cores = (batch b, seq half) pairs. Each core holds X_b fully
(needed for the hid factor) plus a host-transposed copy of its own seq
half (X_b[half].T, needed for the seq factor), computes
    hid_embT = (hid_W*cp) @ X_b          [R, H]
    seq_embT = seq_W @ X_b[half].T       [R, S/2]
    out_half = seq_embT.T @ hid_embT     [S/2, H]
and writes its [512, 1024] slice of the output.

Matmuls run in FP32R (fp32 rounded-to-nearest at 12 mantissa bits, PE
streams it at full rate). Inputs are pre-rounded to the FP32R bit format
on the host so the device does no conversion work.
"""

import numpy as np

B, S, H, R = 4, 1024, 1024, 32
N_CORES = 8
SH = S // 2   # seq rows per core
KT = S // 128  # k-tiles over the contraction dims

_compiled = {}


def _np_fallback(hidden_states, all_indices, seq_W, hid_W, cp_weight):
    seq_emb = np.einsum("bsh,rh->bsr", hidden_states, seq_W)
    hid_emb = np.einsum("bsh,rs->bhr", hidden_states, hid_W)
    s_idx = all_indices[:, 0].astype(np.int64)
    h_idx = all_indices[:, 1].astype(np.int64)
    g_seq = seq_emb[:, s_idx, :]
    g_hid = hid_emb[:, h_idx, :]
    out = np.einsum("bnr,bnr,r->bn", g_seq, g_hid, cp_weight[0])
    return out.reshape(B, S, H).astype(np.float32)


def _round_f32r(a):
    """Round fp32 to the FP32R format (RNE at 12 mantissa bits), bit-exact
    with the device's own fp32->fp32r conversion."""
    b = np.ascontiguousarray(a, dtype=np.float32).view(np.uint32)
    r = (b + np.uint32(0x7FF) + ((b >> np.uint32(12)) & np.uint32(1))) \
        & np.uint32(0xFFFFF000)
    return r.view(np.float32)


def _wtile(w):
    """[K, R] f32 -> [128, KT*R] tile layout, partition-contiguous."""
    return np.ascontiguousarray(
        w.reshape(KT, 128, R).transpose(1, 0, 2).reshape(128, KT * R))


def _build_program():
    import concourse.mybir as mybir
    import concourse.tile as tile
    from concourse import bacc

    f32 = mybir.dt.float32
    f32r = mybir.dt.float32r

    nc = bacc.Bacc("TRN2", target_bir_lowering=False, debug=False,
                   num_devices=N_CORES)

    x_d = nc.dram_tensor("x", [S, H], f32r, kind="ExternalInput")
    xt_d = nc.dram_tensor("xt", [H, SH], f32r, kind="ExternalInput")
    sw_d = nc.dram_tensor("sw", [128, KT * R], f32r, kind="ExternalInput")
    hw_d = nc.dram_tensor("hw", [128, KT * R], f32r, kind="ExternalInput")
    out_d = nc.dram_tensor("out", [SH, H], f32, kind="ExternalOutput")

    with tile.TileContext(nc) as tc:
        with (
            tc.tile_pool(name="xp", bufs=1) as xp,
            tc.tile_pool(name="wp", bufs=1) as wp,
            tc.tile_pool(name="fp", bufs=1) as fp,
            tc.tile_pool(name="op", bufs=3) as op,
            tc.tile_pool(name="fps", bufs=1, space="PSUM") as fps,
            tc.tile_pool(name="ops", bufs=3, space="PSUM") as ops,
        ):
            # weights first (small, gate the first matmuls)
            sw_t = wp.tile([128, KT, R], f32r)
            nc.sync.dma_start(sw_t[:], sw_d.rearrange("p (t r) -> p t r", t=KT))
            hw_t = wp.tile([128, KT, R], f32r)
            nc.sync.dma_start(hw_t[:], hw_d.rearrange("p (t r) -> p t r", t=KT))

            x_t = []
            for k in range(KT):
                xk = xp.tile([128, H], f32r, name=f"x{k}")
                nc.sync.dma_start(xk[:], x_d[k * 128:(k + 1) * 128, :])
                x_t.append(xk)
            xt_t = []
            for k in range(KT):
                xtk = xp.tile([128, SH], f32r, name=f"xt{k}")
                nc.sync.dma_start(xtk[:], xt_d[k * 128:(k + 1) * 128, :])
                xt_t.append(xtk)

            # hid_embT[r, h] = sum_s hid_wT[s, r] * x[s, h]
            hid_sb = fp.tile([R, H], f32r)
            for n in range(2):
                hid_ps = fps.tile([R, 512], f32, name="hid_ps", bufs=2)
                for k in range(KT):
                    nc.tensor.matmul(
                        hid_ps[:],
                        hw_t[:, k, :],
                        x_t[k][:, n * 512:(n + 1) * 512],
                        start=(k == 0),
                        stop=(k == KT - 1),
                    )
                nc.vector.tensor_copy(hid_sb[:, n * 512:(n + 1) * 512], hid_ps[:])

            # seq_embT[r, s] = sum_h seq_wT[h, r] * xt[h, s]
            seq_ps = fps.tile([R, SH], f32)
            for k in range(KT):
                nc.tensor.matmul(
                    seq_ps[:],
                    sw_t[:, k, :],
                    xt_t[k][:],
                    start=(k == 0),
                    stop=(k == KT - 1),
                )
            seq_sb = fp.tile([R, SH], f32r)
            nc.vector.tensor_copy(seq_sb[:], seq_ps[:])

            # out[s, h] = sum_r seq_embT[r, s] * hid_embT[r, h]
            for m in range(SH // 128):
                o_sb = op.tile([128, H], f32, name="o_sb")
                for n in range(2):
                    o_ps = ops.tile([128, 512], f32, name="o_ps")
                    nc.tensor.matmul(
                        o_ps[:],
                        seq_sb[:, m * 128:(m + 1) * 128],
                        hid_sb[:, n * 512:(n + 1) * 512],
                    )
                    nc.vector.tensor_copy(o_sb[:, n * 512:(n + 1) * 512], o_ps[:])
                nc.sync.dma_start(out_d[m * 128:(m + 1) * 128, :], o_sb[:])

    nc.compile()
    return nc


def _get_program():
    if "nc" not in _compiled:
        _compiled["nc"] = _build_program()
    return _compiled["nc"]


def _make_in_maps(hidden_states, seq_W, hid_W, cp_weight):
    swT = _round_f32r(_wtile(np.ascontiguousarray(seq_W.T)))
    hwT = _round_f32r(_wtile(
        np.ascontiguousarray((hid_W * cp_weight[0][:, None]).T)))
    in_maps = []
    for c in range(N_CORES):
        b, half = divmod(c, 2)
        xb = _round_f32r(hidden_states[b])
        in_maps.append({
            "x": xb,
            "xt": np.ascontiguousarray(xb[half * SH:(half + 1) * SH, :].T),
            "sw": swT,
            "hw": hwT,
        })
    return in_maps


def kernel(hidden_states, all_indices, seq_W, hid_W, cp_weight):
    hidden_states = np.asarray(hidden_states, dtype=np.float32)
    seq_W = np.asarray(seq_W, dtype=np.float32)
    hid_W = np.asarray(hid_W, dtype=np.float32)
    cp_weight = np.asarray(cp_weight, dtype=np.float32)
    idx = np.asarray(all_indices)

    # The reference's all_indices is always the full cartesian grid; verify
    # cheaply and fall back to a host path if ever not.
    n = np.arange(S * H, dtype=idx.dtype)
    if idx.shape != (S * H, 2) or not (
        np.array_equal(idx[:, 0], n // H) and np.array_equal(idx[:, 1], n % H)
    ):
        return _np_fallback(hidden_states, idx, seq_W, hid_W, cp_weight)

    from concourse.bass_utils import run_bass_kernel_spmd

    nc = _get_program()
    in_maps = _make_in_maps(hidden_states, seq_W, hid_W, cp_weight)
    res = run_bass_kernel_spmd(nc, in_maps, list(range(N_CORES)))

    out = np.empty((B, S, H), dtype=np.float32)
    for c in range(N_CORES):
        b, half = divmod(c, 2)
        out[b, half * SH:(half + 1) * SH, :] = res.results[c]["out"]
    return out
